# revision 14
# baseline (speedup 1.0000x reference)
"""Pointer-generator decoder kernel for Trainium2 (8 NeuronCores, vocab-sharded).

out = log(p_gen * softmax(LN(x) @ proj_w.T + proj_b) + (1 - p_gen) * enc + 1e-12)
enc = scatter-add of attention mass onto source token ids. Also returns p_gen.

Sharding: tensor-parallel over the vocab dim (V=50000 -> 6250 per core).
Each core computes its vocab shard of the big projection, local sum-exp,
one AllReduce(add) of the per-token softmax denominators, then the final
log-mix for its shard. The scatter-add copy distribution is realized with
two small matmuls against host-built 0/1 selection matrices (G16 gathers +
collision-sums attention columns into <=32 compact slots per 512-wide vocab
tile; Sel16 scatters the compact columns back to dense vocab positions), so
the program is input-independent and compiled once.
"""

import os
import numpy as np

import concourse.bass as bass
import concourse.tile as tile
import concourse.mybir as mybir
from concourse.bass_utils import run_bass_kernel_spmd

F = mybir.ActivationFunctionType
ALU = mybir.AluOpType
FP32 = mybir.dt.float32
BF16 = mybir.dt.bfloat16
FP16 = mybir.dt.float16
I16 = mybir.dt.int16

B, T, S, D, V = 8, 128, 400, 512, 50000
NCORES = 8
VS = V // NCORES          # 6250 vocab per core
VTW = 512                 # vocab tile width (PSUM bank)
VT_WIDTHS = [512] * 12 + [106]
NVT = len(VT_WIDTHS)      # 13
GRP = 4                   # vocab tiles per p2 group (chunk = 2048)
NGRP = (NVT + GRP - 1) // GRP   # 4 groups: [4,4,4,1]
RSLOT = 32                # compact slots per vocab tile
SPAD = 512                # padded source-position axis (S=400 -> 512)
EPS_LN = 1e-6
EPS_MIX = 1e-12

_CACHE = {}
last_results = None       # test harness reads exec_time_ns off this


def _split_big_waits(nc, max_waits=1):
    """This compiler build allows only one sync-wait command per instruction;
    hoist excess waits onto same-engine NOPs placed just before."""
    for f in nc.m.functions:
        for blk in f.blocks:
            new_list = []
            for inst in blk.instructions:
                si = getattr(inst, "sync_info", None)
                waits = list(si.on_wait) if si and si.on_wait else []
                if len(waits) > max_waits:
                    excess, keep = waits[:-max_waits], waits[-max_waits:]
                    for off in range(0, len(excess), max_waits):
                        chunk = excess[off:off + max_waits]
                        nop = mybir.InstNoOp(
                            name=nc.get_next_instruction_name(),
                            ins=[], outs=[], engine=inst.engine)
                        nop.sync_info = mybir.SyncInfo(on_wait=chunk, on_update=[])
                        nc.register_instruction(nop)
                        new_list.append(nop)
                    si.on_wait = keep
                new_list.append(inst)
            blk.instructions[:] = new_list


def _build_program(projb_nonzero):
    nc = bass.Bass()

    # ---- DRAM I/O (per-core shapes; SPMD across 8 cores) ----
    x_d = nc.dram_tensor("x", [B, T, D], FP32, kind="ExternalInput")
    xt_d = nc.dram_tensor("x_t", [B, T, D], FP32, kind="ExternalInput")
    mem_d = nc.dram_tensor("memory", [B, S, D], FP32, kind="ExternalInput")
    attn_d = nc.dram_tensor("attn", [B, T, S], FP32, kind="ExternalInput")
    pwt_d = nc.dram_tensor("pwt", [D, VS], FP16, kind="ExternalInput")
    pb_d = nc.dram_tensor("pb", [VS], FP32, kind="ExternalInput")
    whv_d = nc.dram_tensor("whv", [D], FP32, kind="ExternalInput")
    wsv_d = nc.dram_tensor("wsv", [D], FP32, kind="ExternalInput")
    wxv_d = nc.dram_tensor("wxv", [D], FP32, kind="ExternalInput")
    gate_d = nc.dram_tensor("gatec", [1], FP32, kind="ExternalInput")
    id_d = nc.dram_tensor("identf", [128, 128], FP32, kind="ExternalInput")
    g16_d = nc.dram_tensor("g16", [B, NGRP, 128, 4, GRP * RSLOT], FP16,
                           kind="ExternalInput")
    sel_d = nc.dram_tensor("sel16", [B, NGRP, RSLOT, GRP, VTW], FP32,
                           kind="ExternalInput")

    out_d = nc.dram_tensor("out", [B, T, VS], FP32, kind="ExternalOutput")
    pgen_d = nc.dram_tensor("pgen", [B, T, 1], FP32, kind="ExternalOutput")

    def bcast(handle, n, offset=0):
        return bass.AP(tensor=handle, offset=offset, ap=[[0, 128], [1, n]])

    with tile.TileContext(nc) as tc:
        with tc.tile_pool(name="singles", bufs=1) as singles, \
             tc.tile_pool(name="stage", bufs=2) as stage, \
             tc.tile_pool(name="mm", bufs=2) as mmp, \
             tc.tile_pool(name="p2", bufs=2) as p2p, \
             tc.tile_pool(name="p2s", bufs=3) as p2s, \
             tc.tile_pool(name="small", bufs=4) as small, \
             tc.tile_pool(name="psum", bufs=3, space="PSUM") as psum, \
             tc.tile_pool(name="psum_e", bufs=3, space="PSUM") as psum_e, \
             tc.tile_pool(name="psum_s", bufs=2, space="PSUM") as psum_s, \
             tc.tile_pool(name="dram", bufs=1, space="DRAM") as dram:

            # ---------- singles ----------
            ident = singles.tile([128, 128], FP32)
            nc.sync.dma_start(out=ident[:], in_=id_d[:])
            wh_bc = singles.tile([128, D], FP32)
            nc.sync.dma_start(out=wh_bc[:], in_=bcast(whv_d, D))
            ws_bc = singles.tile([128, D], FP32)
            nc.sync.dma_start(out=ws_bc[:], in_=bcast(wsv_d, D))
            wx_bc = singles.tile([128, D], FP32)
            nc.sync.dma_start(out=wx_bc[:], in_=bcast(wxv_d, D))
            gate_sb = singles.tile([128, 1], FP32)
            nc.sync.dma_start(out=gate_sb[:], in_=bcast(gate_d, 1))
            eps_sb = singles.tile([128, 1], FP32)
            nc.vector.memset(eps_sb[:], EPS_MIX)

            a16 = singles.tile([128, B, VS], FP16)        # exp(dec) stash
            xnT = singles.tile([128, 4, B * T], FP16)     # LN(x)^T  [D, tokens]
            attnT = singles.tile([128, B, 4, 128], FP16)  # attn^T   [s, t] per b
            Z = singles.tile([128, B], FP32)
            nc.vector.memset(Z[:], 0.0)
            p_all = singles.tile([128, B], FP32)
            lnp = singles.tile([128, B], FP32)
            lnq = singles.tile([128, B], FP32)
            bias2 = singles.tile([128, B], FP32)   # lnp - lnq - lnZ
            pz = singles.tile([128, B], FP32)      # exp(lnp - lnZ)
            sinv = singles.tile([128, B], FP32)    # exp(-bias2)

            ones_row = singles.tile([1, 128], FP32)
            nc.vector.memset(ones_row[:], 1.0)

            # ---------- prologue: LN, xnT, attnT, p_gen ----------
            for b in range(B):
                x_t = stage.tile([128, D], FP32, tag="xload")
                nc.sync.dma_start(out=x_t[:], in_=x_d[b])
                xt_t = stage.tile([128, D], FP32, tag="xtload")
                nc.sync.dma_start(out=xt_t[:], in_=xt_d[b])
                at_t = stage.tile([128, SPAD], FP32, tag="atload")
                nc.sync.dma_start(out=at_t[:, :S], in_=attn_d[b])
                nc.vector.memset(at_t[:, S:], 0.0)

                # layer norm of x (affine folded into pwt/pb on host)
                st6 = small.tile([128, 6], FP32, tag="st6")
                nc.vector.bn_stats(out=st6[:], in_=x_t[:])
                mv = small.tile([128, 2], FP32, tag="mv")
                nc.vector.bn_aggr(out=mv[:], in_=st6[:])
                sd = small.tile([128, 1], FP32, tag="sd")
                nc.scalar.sqrt(out=sd[:], in_=mv[:, 1:2])
                nc.vector.tensor_scalar_add(out=sd[:], in0=sd[:], scalar1=EPS_LN)
                inv = small.tile([128, 1], FP32, tag="inv")
                nc.vector.reciprocal(out=inv[:], in_=sd[:])
                xn_t = stage.tile([128, D], FP32, tag="xn")
                nc.vector.tensor_scalar(out=xn_t[:], in0=x_t[:],
                                        scalar1=mv[:, 0:1], scalar2=inv[:],
                                        op0=ALU.subtract, op1=ALU.mult)

                # xn^T and attn^T via PE transpose
                for k in range(4):
                    tp = psum_s.tile([128, 128], FP32, space="PSUM", tag="misc")
                    nc.tensor.transpose(out=tp[:], in_=xn_t[:, 128 * k:128 * (k + 1)],
                                        identity=ident[:])
                    nc.vector.tensor_copy(out=xnT[:, k, T * b:T * (b + 1)], in_=tp[:])
                atT_f = stage.tile([128, 4, 128], FP32, tag="atTf")
                for j in range(4):
                    tp = psum_s.tile([128, 128], FP32, space="PSUM", tag="misc")
                    nc.tensor.transpose(out=tp[:], in_=at_t[:, 128 * j:128 * (j + 1)],
                                        identity=ident[:])
                    nc.vector.tensor_copy(out=atT_f[:, j, :], in_=tp[:])
                    nc.vector.tensor_scalar_mul(out=attnT[:, b, j, :], in0=tp[:],
                                                scalar1=4096.0)

                # h_t = attn @ memory, then p_gen gate pieces
                ht = psum.tile([128, D], FP32, space="PSUM", tag="bank")
                for j in range(4):
                    m_t = stage.tile([128, D], FP32, tag="mload")
                    rows = 128 if j < 3 else S - 384
                    nc.sync.dma_start(out=m_t[:rows, :],
                                      in_=mem_d[b, 128 * j:128 * j + rows])
                    nc.tensor.matmul(out=ht[:], lhsT=atT_f[:, j, :], rhs=m_t[:],
                                     start=(j == 0), stop=(j == 3))

                dump = stage.tile([128, D], FP32, tag="xn")
                hw = small.tile([128, 1], FP32, tag="hw")
                nc.vector.tensor_tensor(out=dump[:], in0=ht[:], in1=wh_bc[:], op=ALU.mult)
                nc.vector.tensor_reduce(out=hw[:], in_=dump[:], axis=mybir.AxisListType.X, op=ALU.add)
                xs = small.tile([128, 1], FP32, tag="xs")
                nc.vector.tensor_tensor(out=dump[:], in0=x_t[:], in1=ws_bc[:], op=ALU.mult)
                nc.vector.tensor_reduce(out=xs[:], in_=dump[:], axis=mybir.AxisListType.X, op=ALU.add)
                xts = small.tile([128, 1], FP32, tag="xts")
                nc.vector.tensor_tensor(out=dump[:], in0=xt_t[:], in1=wx_bc[:], op=ALU.mult)
                nc.vector.tensor_reduce(out=xts[:], in_=dump[:], axis=mybir.AxisListType.X, op=ALU.add)

                logit = small.tile([128, 1], FP32, tag="logit")
                nc.vector.tensor_tensor(out=logit[:], in0=hw[:], in1=xs[:], op=ALU.add)
                nc.vector.tensor_tensor(out=logit[:], in0=logit[:], in1=xts[:], op=ALU.add)
                nc.scalar.activation(out=p_all[:, b:b + 1], in_=logit[:],
                                     func=F.Sigmoid, bias=gate_sb[:])
                pc = small.tile([128, 1], FP32, tag="pc")
                nc.vector.tensor_scalar_max(out=pc[:], in0=p_all[:, b:b + 1], scalar1=1e-30)
                nc.scalar.activation(out=lnp[:, b:b + 1], in_=pc[:], func=F.Ln)
                qc = small.tile([128, 1], FP32, tag="qc")
                nc.vector.tensor_scalar(out=qc[:], in0=p_all[:, b:b + 1],
                                        scalar1=-1.0, scalar2=1.0,
                                        op0=ALU.mult, op1=ALU.add)
                nc.vector.tensor_scalar_max(out=qc[:], in0=qc[:], scalar1=1e-30)
                nc.scalar.activation(out=lnq[:, b:b + 1], in_=qc[:], func=F.Ln)
                nc.sync.dma_start(out=pgen_d[b], in_=p_all[:, b:b + 1])

            # ---------- phase 1: dec = xn @ pwt, exp stash + sumexp ----------
            for g2 in range(0, NVT, 2):
                vts = [v for v in (g2, g2 + 1) if v < NVT]
                pw_tiles = {}
                for v in vts:
                    off = VTW * v
                    w = VT_WIDTHS[v]
                    pwt_t = mmp.tile([128, 4, VTW], FP16, tag="pwt")
                    for k in range(4):
                        nc.sync.dma_start(out=pwt_t[:, k, :w],
                                          in_=pwt_d[128 * k:128 * (k + 1), off:off + w])
                    pw_tiles[v] = pwt_t
                for b in range(B):
                    banks = {}
                    for v in vts:
                        banks[v] = psum.tile([128, VTW], FP32, space="PSUM", tag="bank", name=f"bank{v % 2}")
                    for k in range(4):
                        for v in vts:
                            w = VT_WIDTHS[v]
                            nc.tensor.matmul(
                                out=banks[v][:, :w],
                                lhsT=xnT[:, k, T * b:T * (b + 1)],
                                rhs=pw_tiles[v][:, k, :w],
                                start=(k == 0), stop=(k == 3))
                    for v in vts:
                        w = VT_WIDTHS[v]
                        off = VTW * v
                        if projb_nonzero:
                            pbb = stage.tile([128, VTW], FP32, tag="pbb")
                            nc.sync.dma_start(out=pbb[:, :w], in_=bcast(pb_d, w, offset=off))
                            nc.vector.tensor_tensor(out=banks[v][:, :w], in0=banks[v][:, :w],
                                                    in1=pbb[:, :w], op=ALU.add)
                        sacc = small.tile([128, 1], FP32, tag="sacc")
                        nc.scalar.activation(out=a16[:, b, off:off + w], in_=banks[v][:, :w],
                                             func=F.Exp, accum_out=sacc[:])
                        nc.vector.tensor_tensor(out=Z[:, b:b + 1], in0=Z[:, b:b + 1],
                                                in1=sacc[:], op=ALU.add)

            # ---------- AllReduce of sum-exp, then per-token scales ----------
            cc_in = dram.tile([128, B], FP32)
            cc_out = dram.tile([128, B], FP32)
            nc.sync.dma_start(out=cc_in[:], in_=Z[:])
            nc.gpsimd.collective_compute(
                "AllReduce", ALU.add, replica_groups=[list(range(NCORES))],
                ins=[cc_in[:].opt()], outs=[cc_out[:].opt()])
            Zg = singles.tile([128, B], FP32)
            nc.sync.dma_start(out=Zg[:], in_=cc_out[:])

            lnZ = singles.tile([128, B], FP32)
            nc.scalar.activation(out=lnZ[:], in_=Zg[:], func=F.Ln)
            nc.vector.tensor_tensor(out=bias2[:], in0=lnp[:], in1=lnq[:], op=ALU.subtract)
            nc.vector.tensor_tensor(out=bias2[:], in0=bias2[:], in1=lnZ[:], op=ALU.subtract)
            # pz = exp(lnp - lnZ), sinv = exp(-bias2)
            tmp8 = singles.tile([128, B], FP32)
            nc.vector.tensor_tensor(out=tmp8[:], in0=lnp[:], in1=lnZ[:], op=ALU.subtract)
            nc.scalar.activation(out=pz[:], in_=tmp8[:], func=F.Exp)
            # sinv = exp(-bias2)/4096: compensates the 2^12 scaling baked into
            # attnT to keep tiny attention weights in fp16 normal range
            nc.vector.tensor_scalar(out=tmp8[:], in0=bias2[:], scalar1=-1.0,
                                    scalar2=-12.0 * 0.6931471805599453,
                                    op0=ALU.mult, op1=ALU.add)
            nc.scalar.activation(out=sinv[:], in_=tmp8[:], func=F.Exp)

            # ---------- phase 2: mix + log, per (batch, group-of-4-vtiles) ----------
            for b in range(B):
                # sbc[p, t] = sinv[t, b]: transpose the column to a row via PE
                # (out[1,128] = sinv_col.T @ I), then broadcast across partitions
                # with a K=1 ones matmul.
                srow_ps = psum_s.tile([128, 128], FP32, space="PSUM", tag="misc", name="srow_ps")
                nc.tensor.matmul(out=srow_ps[:1, :], lhsT=sinv[:, b:b + 1], rhs=ident[:],
                                 start=True, stop=True)
                srow = small.tile([1, 128], FP32, tag="srow_sb")
                nc.vector.tensor_copy(out=srow[:], in_=srow_ps[:1, :])
                sbc_ps = psum_s.tile([128, 128], FP32, space="PSUM", tag="misc", name="sbc_ps")
                nc.tensor.matmul(out=sbc_ps[:], lhsT=ones_row[:], rhs=srow[:],
                                 start=True, stop=True)
                sbc = small.tile([128, 128], FP32, tag="sbc_sb")
                nc.vector.tensor_copy(out=sbc[:], in_=sbc_ps[:])

                for g in range(NGRP):
                    goff = 2048 * g
                    vts = [v for v in range(GRP * g, min(GRP * (g + 1), NVT))]
                    cw = sum(VT_WIDTHS[v] for v in vts)

                    g16_t = p2s.tile([128, 4, GRP * RSLOT], FP16, tag="g16")
                    nc.sync.dma_start(out=g16_t[:], in_=g16_d[b, g])
                    sel_t = p2p.tile([RSLOT, GRP, VTW], FP32, tag="sel")
                    nc.sync.dma_start(out=sel_t[:], in_=sel_d[b, g])

                    # M1: compact^T [slots, t] = G16^T @ attn^T
                    cT_ps = psum_s.tile([GRP * RSLOT, 128], FP32, space="PSUM", tag="misc", name="cT_ps")
                    for k in range(4):
                        nc.tensor.matmul(out=cT_ps[:], lhsT=g16_t[:, k, :],
                                         rhs=attnT[:, b, k, :],
                                         start=(k == 0), stop=(k == 3))
                    # scale by sinv[t] during PSUM->SBUF copy (t is the free axis);
                    # split to 4 x [32,128] so M2 operands start at partition 0
                    cT_sb = p2s.tile([RSLOT, GRP, 128], FP32, tag="cTsb")
                    for vi in range(GRP):
                        nc.vector.tensor_tensor(
                            out=cT_sb[:, vi, :],
                            in0=cT_ps[RSLOT * vi:RSLOT * (vi + 1), :],
                            in1=sbc[:RSLOT, :], op=ALU.mult)

                    # m = exp(dec) + enc', per vocab tile (enc' carries q/pz);
                    # then out = ln(pz*m + eps) in place.
                    A_t = p2p.tile([128, 2048], FP32, tag="A")
                    for vi, v in enumerate(vts):
                        w = VT_WIDTHS[v]
                        enc_ps = psum_e.tile([128, VTW], FP32, space="PSUM", tag="enc")
                        nc.tensor.matmul(out=enc_ps[:, :w],
                                         lhsT=cT_sb[:, vi, :],
                                         rhs=sel_t[:, vi, :w],
                                         start=True, stop=True)
                        sl = slice(VTW * vi, VTW * vi + w)
                        nc.vector.tensor_tensor(out=A_t[:, sl],
                                                in0=a16[:, b, goff + VTW * vi:goff + VTW * vi + w],
                                                in1=enc_ps[:, :w], op=ALU.add)
                    nc.scalar.activation(out=A_t[:, :cw], in_=A_t[:, :cw], func=F.Ln,
                                         scale=pz[:, b:b + 1], bias=eps_sb[:])
                    nc.sync.dma_start(out=out_d[b][:, goff:goff + cw], in_=A_t[:, :cw])

    _split_big_waits(nc)
    return nc


def _host_prep(inputs):
    x = np.ascontiguousarray(inputs["x"], np.float32)
    x_t = np.ascontiguousarray(inputs["x_t"], np.float32)
    memory = np.ascontiguousarray(inputs["memory"], np.float32)
    attn = np.ascontiguousarray(inputs["attn_weights"], np.float32)
    src = np.asarray(inputs["src"])
    norm_a = np.asarray(inputs["norm_a"], np.float32)
    norm_b = np.asarray(inputs["norm_b"], np.float32)
    pw = np.asarray(inputs["proj_w"], np.float32)
    pb = np.asarray(inputs["proj_b"], np.float32)

    # fold LN affine into the projection:  dec = LNraw(x) @ (a*pw)^T + (pb + pw@b)
    pw_eff_T = (pw * norm_a[None, :]).T          # [D, V]
    pb_eff = pb + pw @ norm_b                    # [V]
    projb_nonzero = bool(np.any(pb_eff != 0.0))

    gatec = np.array(
        [float(inputs["wh_b"][0]) + float(inputs["ws_b"][0])
         + float(inputs["wx_b"][0]) + float(inputs["bptr"][0, 0])], np.float32)
    identf = np.eye(128, dtype=np.float32)

    # per-core selection matrices for the copy distribution
    g16_all, sel_all, pwt_all, pb_all = [], [], [], []
    for c in range(NCORES):
        lo = c * VS
        g16 = np.zeros((B, NGRP, 128, 4, GRP * RSLOT), np.float32)
        sel = np.zeros((B, NGRP, RSLOT, GRP, VTW), np.float32)
        for b in range(B):
            ids = src[b]
            msk = (ids >= lo) & (ids < lo + VS)
            s_idx = np.nonzero(msk)[0]
            loc = ids[s_idx] - lo
            for g in range(NGRP):
                vts = range(GRP * g, min(GRP * (g + 1), NVT))
                for vi, v in enumerate(vts):
                    vlo, vhi = VTW * v, VTW * v + VT_WIDTHS[v]
                    sub = (loc >= vlo) & (loc < vhi)
                    if not np.any(sub):
                        continue
                    ss = s_idx[sub]
                    ll = loc[sub] - vlo
                    uniq, inv = np.unique(ll, return_inverse=True)
                    if len(uniq) > RSLOT:
                        raise RuntimeError(
                            f"slot overflow: {len(uniq)} > {RSLOT}")
                    col = RSLOT * vi + inv
                    g16[b, g, ss % 128, ss // 128, col] = 1.0
                    sel[b, g, np.arange(len(uniq)), vi, uniq] = 1.0
        g16_all.append(g16.astype(np.float32))
        sel_all.append(sel.astype(np.float32))
        pwt_all.append(np.ascontiguousarray(pw_eff_T[:, lo:lo + VS]))
        pb_all.append(np.ascontiguousarray(pb_eff[lo:lo + VS]))

    in_maps = []
    for c in range(NCORES):
        in_maps.append({
            "x": x, "x_t": x_t, "memory": memory, "attn": attn,
            "pwt": pwt_all[c].astype(np.float16), "pb": pb_all[c],
            "whv": np.ascontiguousarray(inputs["wh_w"][0], np.float32),
            "wsv": np.ascontiguousarray(inputs["ws_w"][0], np.float32),
            "wxv": np.ascontiguousarray(inputs["wx_w"][0], np.float32),
            "gatec": gatec, "identf": identf,
            "g16": g16_all[c].astype(np.float16),
            "sel16": sel_all[c],
        })
    return in_maps, projb_nonzero


def kernel(**inputs):
    global last_results
    in_maps, projb_nonzero = _host_prep(inputs)

    key = ("prog", projb_nonzero)
    if key not in _CACHE:
        _CACHE[key] = _build_program(projb_nonzero)
    nc = _CACHE[key]

    trace = bool(os.environ.get("TRN_KERNEL_TRACE"))
    res = run_bass_kernel_spmd(nc, in_maps, list(range(NCORES)), trace=trace)
    last_results = res

    out = np.empty((B, T, V), np.float32)
    for c in range(NCORES):
        out[:, :, c * VS:(c + 1) * VS] = res.results[c]["out"]
    p_gen = np.ascontiguousarray(res.results[0]["pgen"], np.float32)
    return out, p_gen


# revision 15
# speedup vs baseline: 1.0640x; 1.0640x over previous
"""Pointer-generator decoder kernel for Trainium2 (8 NeuronCores, vocab-sharded).

out = log(p_gen * softmax(LN(x) @ proj_w.T + proj_b) + (1 - p_gen) * enc + 1e-12)
enc = scatter-add of attention mass onto source token ids. Also returns p_gen.

Sharding: tensor-parallel over the vocab dim (V=50000 -> 6250 per core).
Each core computes its vocab shard of the big projection, local sum-exp,
one AllReduce(add) of the per-token softmax denominators, then the final
log-mix for its shard. The scatter-add copy distribution is realized with
two small matmuls against host-built 0/1 selection matrices (G16 gathers +
collision-sums attention columns into <=32 compact slots per 512-wide vocab
tile; Sel16 scatters the compact columns back to dense vocab positions), so
the program is input-independent and compiled once.
"""

import os
import numpy as np

import concourse.bass as bass
import concourse.tile as tile
import concourse.mybir as mybir
from concourse.bass_utils import run_bass_kernel_spmd

F = mybir.ActivationFunctionType
ALU = mybir.AluOpType
FP32 = mybir.dt.float32
BF16 = mybir.dt.bfloat16
FP16 = mybir.dt.float16
I16 = mybir.dt.int16

B, T, S, D, V = 8, 128, 400, 512, 50000
NCORES = 8
VS = V // NCORES          # 6250 vocab per core
VTW = 512                 # vocab tile width (PSUM bank)
VT_WIDTHS = [512] * 12 + [106]
NVT = len(VT_WIDTHS)      # 13
GRP = 4                   # vocab tiles per p2 group (chunk = 2048)
NGRP = (NVT + GRP - 1) // GRP   # 4 groups: [4,4,4,1]
RSLOT = 32                # compact slots per vocab tile
SPAD = 512                # padded source-position axis (S=400 -> 512)
EPS_LN = 1e-6
EPS_MIX = 1e-12

_CACHE = {}
last_results = None       # test harness reads exec_time_ns off this


def _split_big_waits(nc, max_waits=1):
    """This compiler build allows only one sync-wait command per instruction;
    hoist excess waits onto same-engine NOPs placed just before."""
    for f in nc.m.functions:
        for blk in f.blocks:
            new_list = []
            for inst in blk.instructions:
                si = getattr(inst, "sync_info", None)
                waits = list(si.on_wait) if si and si.on_wait else []
                if len(waits) > max_waits:
                    excess, keep = waits[:-max_waits], waits[-max_waits:]
                    for off in range(0, len(excess), max_waits):
                        chunk = excess[off:off + max_waits]
                        nop = mybir.InstNoOp(
                            name=nc.get_next_instruction_name(),
                            ins=[], outs=[], engine=inst.engine)
                        nop.sync_info = mybir.SyncInfo(on_wait=chunk, on_update=[])
                        nc.register_instruction(nop)
                        new_list.append(nop)
                    si.on_wait = keep
                new_list.append(inst)
            blk.instructions[:] = new_list


def _build_program(projb_nonzero):
    nc = bass.Bass()

    # ---- DRAM I/O (per-core shapes; SPMD across 8 cores) ----
    x_d = nc.dram_tensor("x", [B, T, D], FP32, kind="ExternalInput")
    xt_d = nc.dram_tensor("x_t", [B, T, D], FP32, kind="ExternalInput")
    mem_d = nc.dram_tensor("memory", [B, S, D], FP32, kind="ExternalInput")
    attn_d = nc.dram_tensor("attn", [B, T, S], FP32, kind="ExternalInput")
    pwt_d = nc.dram_tensor("pwt", [D, VS], FP16, kind="ExternalInput")
    pb_d = nc.dram_tensor("pb", [VS], FP32, kind="ExternalInput")
    whv_d = nc.dram_tensor("whv", [D], FP32, kind="ExternalInput")
    wsv_d = nc.dram_tensor("wsv", [D], FP32, kind="ExternalInput")
    wxv_d = nc.dram_tensor("wxv", [D], FP32, kind="ExternalInput")
    gate_d = nc.dram_tensor("gatec", [1], FP32, kind="ExternalInput")
    id_d = nc.dram_tensor("identf", [128, 128], FP32, kind="ExternalInput")
    g16_d = nc.dram_tensor("g16", [B, NGRP, 128, 4, GRP * RSLOT], FP16,
                           kind="ExternalInput")
    sel_d = nc.dram_tensor("sel16", [B, NGRP, RSLOT, GRP, VTW], FP32,
                           kind="ExternalInput")

    out_d = nc.dram_tensor("out", [B, T, VS], FP32, kind="ExternalOutput")
    pgen_d = nc.dram_tensor("pgen", [B, T, 1], FP32, kind="ExternalOutput")

    def bcast(handle, n, offset=0):
        return bass.AP(tensor=handle, offset=offset, ap=[[0, 128], [1, n]])

    with tile.TileContext(nc) as tc:
        with tc.tile_pool(name="singles", bufs=1) as singles, \
             tc.tile_pool(name="stage", bufs=2) as stage, \
             tc.tile_pool(name="mm", bufs=3) as mmp, \
             tc.tile_pool(name="p2", bufs=2) as p2p, \
             tc.tile_pool(name="p2s", bufs=3) as p2s, \
             tc.tile_pool(name="small", bufs=4) as small, \
             tc.tile_pool(name="psum", bufs=3, space="PSUM") as psum, \
             tc.tile_pool(name="psum_e", bufs=3, space="PSUM") as psum_e, \
             tc.tile_pool(name="psum_s", bufs=2, space="PSUM") as psum_s, \
             tc.tile_pool(name="dram", bufs=1, space="DRAM") as dram:

            # ---------- singles ----------
            ident = singles.tile([128, 128], FP32)
            nc.sync.dma_start(out=ident[:], in_=id_d[:])
            wh_bc = singles.tile([128, D], FP32)
            nc.sync.dma_start(out=wh_bc[:], in_=bcast(whv_d, D))
            ws_bc = singles.tile([128, D], FP32)
            nc.sync.dma_start(out=ws_bc[:], in_=bcast(wsv_d, D))
            wx_bc = singles.tile([128, D], FP32)
            nc.sync.dma_start(out=wx_bc[:], in_=bcast(wxv_d, D))
            gate_sb = singles.tile([128, 1], FP32)
            nc.sync.dma_start(out=gate_sb[:], in_=bcast(gate_d, 1))
            eps_sb = singles.tile([128, 1], FP32)
            nc.vector.memset(eps_sb[:], EPS_MIX)

            a16 = singles.tile([128, B, VS], FP16)        # exp(dec) stash
            xnT = singles.tile([128, 4, B * T], FP16)     # LN(x)^T  [D, tokens]
            attnT = singles.tile([128, B, 4, 128], FP16)  # attn^T   [s, t] per b
            Z = singles.tile([128, B], FP32)
            nc.vector.memset(Z[:], 0.0)
            p_all = singles.tile([128, B], FP32)
            lnp = singles.tile([128, B], FP32)
            lnq = singles.tile([128, B], FP32)
            bias2 = singles.tile([128, B], FP32)   # lnp - lnq - lnZ
            pz = singles.tile([128, B], FP32)      # exp(lnp - lnZ)
            sinv = singles.tile([128, B], FP32)    # exp(-bias2)

            ones_row = singles.tile([1, 128], FP32)
            nc.vector.memset(ones_row[:], 1.0)

            # ---------- prologue: LN, xnT, attnT, p_gen ----------
            for b in range(B):
                x_t = stage.tile([128, D], FP32, tag="xload")
                nc.sync.dma_start(out=x_t[:], in_=x_d[b])
                xt_t = stage.tile([128, D], FP32, tag="xtload")
                nc.sync.dma_start(out=xt_t[:], in_=xt_d[b])
                at_t = stage.tile([128, SPAD], FP32, tag="atload")
                nc.sync.dma_start(out=at_t[:, :S], in_=attn_d[b])
                nc.vector.memset(at_t[:, S:], 0.0)

                # layer norm of x (affine folded into pwt/pb on host)
                st6 = small.tile([128, 6], FP32, tag="st6")
                nc.vector.bn_stats(out=st6[:], in_=x_t[:])
                mv = small.tile([128, 2], FP32, tag="mv")
                nc.vector.bn_aggr(out=mv[:], in_=st6[:])
                sd = small.tile([128, 1], FP32, tag="sd")
                nc.scalar.sqrt(out=sd[:], in_=mv[:, 1:2])
                nc.vector.tensor_scalar_add(out=sd[:], in0=sd[:], scalar1=EPS_LN)
                inv = small.tile([128, 1], FP32, tag="inv")
                nc.vector.reciprocal(out=inv[:], in_=sd[:])
                xn_t = stage.tile([128, D], FP32, tag="xn")
                nc.vector.tensor_scalar(out=xn_t[:], in0=x_t[:],
                                        scalar1=mv[:, 0:1], scalar2=inv[:],
                                        op0=ALU.subtract, op1=ALU.mult)

                # xn^T and attn^T via PE transpose
                for k in range(4):
                    tp = psum_s.tile([128, 128], FP32, space="PSUM", tag="misc")
                    nc.tensor.transpose(out=tp[:], in_=xn_t[:, 128 * k:128 * (k + 1)],
                                        identity=ident[:])
                    nc.vector.tensor_copy(out=xnT[:, k, T * b:T * (b + 1)], in_=tp[:])
                atT_f = stage.tile([128, 4, 128], FP32, tag="atTf")
                for j in range(4):
                    tp = psum_s.tile([128, 128], FP32, space="PSUM", tag="misc")
                    nc.tensor.transpose(out=tp[:], in_=at_t[:, 128 * j:128 * (j + 1)],
                                        identity=ident[:])
                    nc.vector.tensor_copy(out=atT_f[:, j, :], in_=tp[:])
                    nc.vector.tensor_scalar_mul(out=attnT[:, b, j, :], in0=tp[:],
                                                scalar1=4096.0)

                # h_t = attn @ memory, then p_gen gate pieces
                ht = psum.tile([128, D], FP32, space="PSUM", tag="bank")
                for j in range(4):
                    m_t = stage.tile([128, D], FP32, tag="mload")
                    rows = 128 if j < 3 else S - 384
                    nc.sync.dma_start(out=m_t[:rows, :],
                                      in_=mem_d[b, 128 * j:128 * j + rows])
                    nc.tensor.matmul(out=ht[:], lhsT=atT_f[:, j, :], rhs=m_t[:],
                                     start=(j == 0), stop=(j == 3))

                dump = stage.tile([128, D], FP32, tag="xn")
                hw = small.tile([128, 1], FP32, tag="hw")
                nc.vector.tensor_tensor(out=dump[:], in0=ht[:], in1=wh_bc[:], op=ALU.mult)
                nc.vector.tensor_reduce(out=hw[:], in_=dump[:], axis=mybir.AxisListType.X, op=ALU.add)
                xs = small.tile([128, 1], FP32, tag="xs")
                nc.vector.tensor_tensor(out=dump[:], in0=x_t[:], in1=ws_bc[:], op=ALU.mult)
                nc.vector.tensor_reduce(out=xs[:], in_=dump[:], axis=mybir.AxisListType.X, op=ALU.add)
                xts = small.tile([128, 1], FP32, tag="xts")
                nc.vector.tensor_tensor(out=dump[:], in0=xt_t[:], in1=wx_bc[:], op=ALU.mult)
                nc.vector.tensor_reduce(out=xts[:], in_=dump[:], axis=mybir.AxisListType.X, op=ALU.add)

                logit = small.tile([128, 1], FP32, tag="logit")
                nc.vector.tensor_tensor(out=logit[:], in0=hw[:], in1=xs[:], op=ALU.add)
                nc.vector.tensor_tensor(out=logit[:], in0=logit[:], in1=xts[:], op=ALU.add)
                nc.scalar.activation(out=p_all[:, b:b + 1], in_=logit[:],
                                     func=F.Sigmoid, bias=gate_sb[:])
                pc = small.tile([128, 1], FP32, tag="pc")
                nc.vector.tensor_scalar_max(out=pc[:], in0=p_all[:, b:b + 1], scalar1=1e-30)
                nc.scalar.activation(out=lnp[:, b:b + 1], in_=pc[:], func=F.Ln)
                qc = small.tile([128, 1], FP32, tag="qc")
                nc.vector.tensor_scalar(out=qc[:], in0=p_all[:, b:b + 1],
                                        scalar1=-1.0, scalar2=1.0,
                                        op0=ALU.mult, op1=ALU.add)
                nc.vector.tensor_scalar_max(out=qc[:], in0=qc[:], scalar1=1e-30)
                nc.scalar.activation(out=lnq[:, b:b + 1], in_=qc[:], func=F.Ln)
                nc.sync.dma_start(out=pgen_d[b], in_=p_all[:, b:b + 1])

            # ---------- phase 1: dec = xn @ pwt, exp stash + sumexp ----------
            for g2 in range(0, NVT, 2):
                vts = [v for v in (g2, g2 + 1) if v < NVT]
                pw_tiles = {}
                for v in vts:
                    off = VTW * v
                    w = VT_WIDTHS[v]
                    pwt_t = mmp.tile([128, 4, VTW], FP16, tag="pwt")
                    for k in range(4):
                        nc.sync.dma_start(out=pwt_t[:, k, :w],
                                          in_=pwt_d[128 * k:128 * (k + 1), off:off + w])
                    pw_tiles[v] = pwt_t
                for b in range(B):
                    banks = {}
                    for v in vts:
                        banks[v] = psum.tile([128, VTW], FP32, space="PSUM", tag="bank", name=f"bank{v % 2}")
                    for k in range(4):
                        for v in vts:
                            w = VT_WIDTHS[v]
                            nc.tensor.matmul(
                                out=banks[v][:, :w],
                                lhsT=xnT[:, k, T * b:T * (b + 1)],
                                rhs=pw_tiles[v][:, k, :w],
                                start=(k == 0), stop=(k == 3))
                    for v in vts:
                        w = VT_WIDTHS[v]
                        off = VTW * v
                        if projb_nonzero:
                            pbb = stage.tile([128, VTW], FP32, tag="pbb")
                            nc.sync.dma_start(out=pbb[:, :w], in_=bcast(pb_d, w, offset=off))
                            nc.vector.tensor_tensor(out=banks[v][:, :w], in0=banks[v][:, :w],
                                                    in1=pbb[:, :w], op=ALU.add)
                        sacc = small.tile([128, 1], FP32, tag="sacc")
                        nc.scalar.activation(out=a16[:, b, off:off + w], in_=banks[v][:, :w],
                                             func=F.Exp, accum_out=sacc[:])
                        nc.vector.tensor_tensor(out=Z[:, b:b + 1], in0=Z[:, b:b + 1],
                                                in1=sacc[:], op=ALU.add)

            # ---------- AllReduce of sum-exp, then per-token scales ----------
            cc_in = dram.tile([128, B], FP32)
            cc_out = dram.tile([128, B], FP32)
            nc.sync.dma_start(out=cc_in[:], in_=Z[:])
            nc.gpsimd.collective_compute(
                "AllReduce", ALU.add, replica_groups=[list(range(NCORES))],
                ins=[cc_in[:].opt()], outs=[cc_out[:].opt()])
            Zg = singles.tile([128, B], FP32)
            nc.sync.dma_start(out=Zg[:], in_=cc_out[:])

            lnZ = singles.tile([128, B], FP32)
            nc.scalar.activation(out=lnZ[:], in_=Zg[:], func=F.Ln)
            nc.vector.tensor_tensor(out=bias2[:], in0=lnp[:], in1=lnq[:], op=ALU.subtract)
            nc.vector.tensor_tensor(out=bias2[:], in0=bias2[:], in1=lnZ[:], op=ALU.subtract)
            # pz = exp(lnp - lnZ), sinv = exp(-bias2)
            tmp8 = singles.tile([128, B], FP32)
            nc.vector.tensor_tensor(out=tmp8[:], in0=lnp[:], in1=lnZ[:], op=ALU.subtract)
            nc.scalar.activation(out=pz[:], in_=tmp8[:], func=F.Exp)
            # sinv = exp(-bias2)/4096: compensates the 2^12 scaling baked into
            # attnT to keep tiny attention weights in fp16 normal range
            nc.vector.tensor_scalar(out=tmp8[:], in0=bias2[:], scalar1=-1.0,
                                    scalar2=-12.0 * 0.6931471805599453,
                                    op0=ALU.mult, op1=ALU.add)
            nc.scalar.activation(out=sinv[:], in_=tmp8[:], func=F.Exp)

            # ---------- phase 2: mix + log, per (batch, group-of-4-vtiles) ----------
            for b in range(B):
                # sbc[p, t] = sinv[t, b]: transpose the column to a row via PE
                # (out[1,128] = sinv_col.T @ I), then broadcast across partitions
                # with a K=1 ones matmul.
                srow_ps = psum_s.tile([128, 128], FP32, space="PSUM", tag="misc", name="srow_ps")
                nc.tensor.matmul(out=srow_ps[:1, :], lhsT=sinv[:, b:b + 1], rhs=ident[:],
                                 start=True, stop=True)
                srow = small.tile([1, 128], FP32, tag="srow_sb")
                nc.vector.tensor_copy(out=srow[:], in_=srow_ps[:1, :])
                sbc_ps = psum_s.tile([128, 128], FP32, space="PSUM", tag="misc", name="sbc_ps")
                nc.tensor.matmul(out=sbc_ps[:], lhsT=ones_row[:], rhs=srow[:],
                                 start=True, stop=True)
                sbc = small.tile([128, 128], FP32, tag="sbc_sb")
                nc.vector.tensor_copy(out=sbc[:], in_=sbc_ps[:])

                for g in range(NGRP):
                    goff = 2048 * g
                    vts = [v for v in range(GRP * g, min(GRP * (g + 1), NVT))]
                    cw = sum(VT_WIDTHS[v] for v in vts)

                    g16_t = p2s.tile([128, 4, GRP * RSLOT], FP16, tag="g16")
                    nc.sync.dma_start(out=g16_t[:], in_=g16_d[b, g])
                    sel_t = p2p.tile([RSLOT, GRP, VTW], FP32, tag="sel")
                    nc.sync.dma_start(out=sel_t[:], in_=sel_d[b, g])

                    # M1: compact^T [slots, t] = G16^T @ attn^T
                    cT_ps = psum_s.tile([GRP * RSLOT, 128], FP32, space="PSUM", tag="misc", name="cT_ps")
                    for k in range(4):
                        nc.tensor.matmul(out=cT_ps[:], lhsT=g16_t[:, k, :],
                                         rhs=attnT[:, b, k, :],
                                         start=(k == 0), stop=(k == 3))
                    # scale by sinv[t] during PSUM->SBUF copy (t is the free axis);
                    # split to 4 x [32,128] so M2 operands start at partition 0
                    cT_sb = p2s.tile([RSLOT, GRP, 128], FP32, tag="cTsb")
                    for vi in range(GRP):
                        nc.vector.tensor_tensor(
                            out=cT_sb[:, vi, :],
                            in0=cT_ps[RSLOT * vi:RSLOT * (vi + 1), :],
                            in1=sbc[:RSLOT, :], op=ALU.mult)

                    # m = exp(dec) + enc', per vocab tile (enc' carries q/pz);
                    # then out = ln(pz*m + eps) in place.
                    A_t = p2p.tile([128, 2048], FP32, tag="A")
                    for vi, v in enumerate(vts):
                        w = VT_WIDTHS[v]
                        enc_ps = psum_e.tile([128, VTW], FP32, space="PSUM", tag="enc")
                        nc.tensor.matmul(out=enc_ps[:, :w],
                                         lhsT=cT_sb[:, vi, :],
                                         rhs=sel_t[:, vi, :w],
                                         start=True, stop=True)
                        sl = slice(VTW * vi, VTW * vi + w)
                        nc.vector.tensor_tensor(out=A_t[:, sl],
                                                in0=a16[:, b, goff + VTW * vi:goff + VTW * vi + w],
                                                in1=enc_ps[:, :w], op=ALU.add)
                    nc.scalar.activation(out=A_t[:, :cw], in_=A_t[:, :cw], func=F.Ln,
                                         scale=pz[:, b:b + 1], bias=eps_sb[:])
                    nc.sync.dma_start(out=out_d[b][:, goff:goff + cw], in_=A_t[:, :cw])

    _split_big_waits(nc)
    return nc


def _host_prep(inputs):
    x = np.ascontiguousarray(inputs["x"], np.float32)
    x_t = np.ascontiguousarray(inputs["x_t"], np.float32)
    memory = np.ascontiguousarray(inputs["memory"], np.float32)
    attn = np.ascontiguousarray(inputs["attn_weights"], np.float32)
    src = np.asarray(inputs["src"])
    norm_a = np.asarray(inputs["norm_a"], np.float32)
    norm_b = np.asarray(inputs["norm_b"], np.float32)
    pw = np.asarray(inputs["proj_w"], np.float32)
    pb = np.asarray(inputs["proj_b"], np.float32)

    # fold LN affine into the projection:  dec = LNraw(x) @ (a*pw)^T + (pb + pw@b)
    pw_eff_T = (pw * norm_a[None, :]).T          # [D, V]
    pb_eff = pb + pw @ norm_b                    # [V]
    projb_nonzero = bool(np.any(pb_eff != 0.0))

    gatec = np.array(
        [float(inputs["wh_b"][0]) + float(inputs["ws_b"][0])
         + float(inputs["wx_b"][0]) + float(inputs["bptr"][0, 0])], np.float32)
    identf = np.eye(128, dtype=np.float32)

    # per-core selection matrices for the copy distribution
    g16_all, sel_all, pwt_all, pb_all = [], [], [], []
    for c in range(NCORES):
        lo = c * VS
        g16 = np.zeros((B, NGRP, 128, 4, GRP * RSLOT), np.float32)
        sel = np.zeros((B, NGRP, RSLOT, GRP, VTW), np.float32)
        for b in range(B):
            ids = src[b]
            msk = (ids >= lo) & (ids < lo + VS)
            s_idx = np.nonzero(msk)[0]
            loc = ids[s_idx] - lo
            for g in range(NGRP):
                vts = range(GRP * g, min(GRP * (g + 1), NVT))
                for vi, v in enumerate(vts):
                    vlo, vhi = VTW * v, VTW * v + VT_WIDTHS[v]
                    sub = (loc >= vlo) & (loc < vhi)
                    if not np.any(sub):
                        continue
                    ss = s_idx[sub]
                    ll = loc[sub] - vlo
                    uniq, inv = np.unique(ll, return_inverse=True)
                    if len(uniq) > RSLOT:
                        raise RuntimeError(
                            f"slot overflow: {len(uniq)} > {RSLOT}")
                    col = RSLOT * vi + inv
                    g16[b, g, ss % 128, ss // 128, col] = 1.0
                    sel[b, g, np.arange(len(uniq)), vi, uniq] = 1.0
        g16_all.append(g16.astype(np.float32))
        sel_all.append(sel.astype(np.float32))
        pwt_all.append(np.ascontiguousarray(pw_eff_T[:, lo:lo + VS]))
        pb_all.append(np.ascontiguousarray(pb_eff[lo:lo + VS]))

    in_maps = []
    for c in range(NCORES):
        in_maps.append({
            "x": x, "x_t": x_t, "memory": memory, "attn": attn,
            "pwt": pwt_all[c].astype(np.float16), "pb": pb_all[c],
            "whv": np.ascontiguousarray(inputs["wh_w"][0], np.float32),
            "wsv": np.ascontiguousarray(inputs["ws_w"][0], np.float32),
            "wxv": np.ascontiguousarray(inputs["wx_w"][0], np.float32),
            "gatec": gatec, "identf": identf,
            "g16": g16_all[c].astype(np.float16),
            "sel16": sel_all[c],
        })
    return in_maps, projb_nonzero


def kernel(**inputs):
    global last_results
    in_maps, projb_nonzero = _host_prep(inputs)

    key = ("prog", projb_nonzero)
    if key not in _CACHE:
        _CACHE[key] = _build_program(projb_nonzero)
    nc = _CACHE[key]

    trace = bool(os.environ.get("TRN_KERNEL_TRACE"))
    res = run_bass_kernel_spmd(nc, in_maps, list(range(NCORES)), trace=trace)
    last_results = res

    out = np.empty((B, T, V), np.float32)
    for c in range(NCORES):
        out[:, :, c * VS:(c + 1) * VS] = res.results[c]["out"]
    p_gen = np.ascontiguousarray(res.results[0]["pgen"], np.float32)
    return out, p_gen


# revision 16
# speedup vs baseline: 1.2926x; 1.2148x over previous
"""Pointer-generator decoder kernel for Trainium2 (8 NeuronCores, vocab-sharded).

out = log(p_gen * softmax(LN(x) @ proj_w.T + proj_b) + (1 - p_gen) * enc + 1e-12)
enc = scatter-add of attention mass onto source token ids. Also returns p_gen.

Sharding: tensor-parallel over the vocab dim (V=50000 -> 6250 per core).
Each core computes its vocab shard of the big projection, local sum-exp,
one AllReduce(add) of the per-token softmax denominators, then the final
log-mix for its shard. The scatter-add copy distribution is realized with
two small matmuls against host-built 0/1 selection matrices (G16 gathers +
collision-sums attention columns into <=32 compact slots per 512-wide vocab
tile; Sel16 scatters the compact columns back to dense vocab positions), so
the program is input-independent and compiled once.
"""

import os
import numpy as np

import concourse.bass as bass
import concourse.tile as tile
import concourse.mybir as mybir
from concourse.bass_utils import run_bass_kernel_spmd

F = mybir.ActivationFunctionType
ALU = mybir.AluOpType
FP32 = mybir.dt.float32
BF16 = mybir.dt.bfloat16
FP16 = mybir.dt.float16
I16 = mybir.dt.int16

B, T, S, D, V = 8, 128, 400, 512, 50000
NCORES = 8
VS = V // NCORES          # 6250 vocab per core
VTW = 512                 # vocab tile width (PSUM bank)
VT_WIDTHS = [512] * 12 + [106]
NVT = len(VT_WIDTHS)      # 13
GRP = 4                   # vocab tiles per p2 group (chunk = 2048)
NGRP = (NVT + GRP - 1) // GRP   # 4 groups: [4,4,4,1]
RSLOT = 32                # compact slots per vocab tile
SPAD = 512                # padded source-position axis (S=400 -> 512)
EPS_LN = 1e-6
EPS_MIX = 1e-12

_CACHE = {}
last_results = None       # test harness reads exec_time_ns off this


def _split_big_waits(nc, max_waits=1):
    """This compiler build allows only one sync-wait command per instruction;
    hoist excess waits onto same-engine NOPs placed just before."""
    for f in nc.m.functions:
        for blk in f.blocks:
            new_list = []
            for inst in blk.instructions:
                si = getattr(inst, "sync_info", None)
                waits = list(si.on_wait) if si and si.on_wait else []
                if len(waits) > max_waits:
                    excess, keep = waits[:-max_waits], waits[-max_waits:]
                    for off in range(0, len(excess), max_waits):
                        chunk = excess[off:off + max_waits]
                        nop = mybir.InstNoOp(
                            name=nc.get_next_instruction_name(),
                            ins=[], outs=[], engine=inst.engine)
                        nop.sync_info = mybir.SyncInfo(on_wait=chunk, on_update=[])
                        nc.register_instruction(nop)
                        new_list.append(nop)
                    si.on_wait = keep
                new_list.append(inst)
            blk.instructions[:] = new_list


def _build_program(projb_nonzero):
    nc = bass.Bass()

    # ---- DRAM I/O (per-core shapes; SPMD across 8 cores) ----
    x_d = nc.dram_tensor("x", [B, T, D], FP32, kind="ExternalInput")
    xt_d = nc.dram_tensor("x_t", [B, T, D], FP32, kind="ExternalInput")
    mem_d = nc.dram_tensor("memory", [B, S, D], FP32, kind="ExternalInput")
    attn_d = nc.dram_tensor("attn", [B, T, S], FP32, kind="ExternalInput")
    pwt_d = nc.dram_tensor("pwt", [D, VS], FP16, kind="ExternalInput")
    pb_d = nc.dram_tensor("pb", [VS], FP32, kind="ExternalInput")
    whv_d = nc.dram_tensor("whv", [D], FP32, kind="ExternalInput")
    wsv_d = nc.dram_tensor("wsv", [D], FP32, kind="ExternalInput")
    wxv_d = nc.dram_tensor("wxv", [D], FP32, kind="ExternalInput")
    gate_d = nc.dram_tensor("gatec", [1], FP32, kind="ExternalInput")
    id_d = nc.dram_tensor("identf", [128, 128], FP32, kind="ExternalInput")
    g16_d = nc.dram_tensor("g16", [B, NGRP, 128, 4, GRP * RSLOT], FP16,
                           kind="ExternalInput")
    sel_d = nc.dram_tensor("sel16", [B, NGRP, RSLOT, GRP, VTW], BF16,
                           kind="ExternalInput")

    out_d = nc.dram_tensor("out", [B, T, VS], FP32, kind="ExternalOutput")
    pgen_d = nc.dram_tensor("pgen", [B, T, 1], FP32, kind="ExternalOutput")

    def bcast(handle, n, offset=0):
        return bass.AP(tensor=handle, offset=offset, ap=[[0, 128], [1, n]])

    with tile.TileContext(nc) as tc:
        with tc.tile_pool(name="singles", bufs=1) as singles, \
             tc.tile_pool(name="stage", bufs=2) as stage, \
             tc.tile_pool(name="mm", bufs=3) as mmp, \
             tc.tile_pool(name="p2", bufs=2) as p2p, \
             tc.tile_pool(name="p2s", bufs=3) as p2s, \
             tc.tile_pool(name="small", bufs=4) as small, \
             tc.tile_pool(name="psum", bufs=3, space="PSUM") as psum, \
             tc.tile_pool(name="psum_e", bufs=3, space="PSUM") as psum_e, \
             tc.tile_pool(name="psum_s", bufs=2, space="PSUM") as psum_s, \
             tc.tile_pool(name="dram", bufs=1, space="DRAM") as dram:

            # ---------- singles ----------
            ident = singles.tile([128, 128], FP32)
            nc.sync.dma_start(out=ident[:], in_=id_d[:])
            wh_bc = singles.tile([128, D], FP32)
            nc.sync.dma_start(out=wh_bc[:], in_=bcast(whv_d, D))
            ws_bc = singles.tile([128, D], FP32)
            nc.sync.dma_start(out=ws_bc[:], in_=bcast(wsv_d, D))
            wx_bc = singles.tile([128, D], FP32)
            nc.sync.dma_start(out=wx_bc[:], in_=bcast(wxv_d, D))
            gate_sb = singles.tile([128, 1], FP32)
            nc.sync.dma_start(out=gate_sb[:], in_=bcast(gate_d, 1))
            eps_sb = singles.tile([128, 1], FP32)
            nc.vector.memset(eps_sb[:], EPS_MIX)

            a16 = singles.tile([128, B, VS], FP16)        # exp(dec) stash
            xnT = singles.tile([128, 4, B * T], FP16)     # LN(x)^T  [D, tokens]
            attnT = singles.tile([128, B, 4, 128], FP16)  # attn^T   [s, t] per b
            Z = singles.tile([128, B], FP32)
            nc.vector.memset(Z[:], 0.0)
            p_all = singles.tile([128, B], FP32)
            lnp = singles.tile([128, B], FP32)
            lnq = singles.tile([128, B], FP32)
            bias2 = singles.tile([128, B], FP32)   # lnp - lnq - lnZ
            pz = singles.tile([128, B], FP32)      # exp(lnp - lnZ)
            sinv = singles.tile([128, B], FP32)    # exp(-bias2)

            ones_row = singles.tile([1, 128], FP32)
            nc.vector.memset(ones_row[:], 1.0)

            # ---------- prologue: LN, xnT, attnT, p_gen ----------
            for b in range(B):
                x_t = stage.tile([128, D], FP32, tag="xload")
                nc.sync.dma_start(out=x_t[:], in_=x_d[b])
                xt_t = stage.tile([128, D], FP32, tag="xtload")
                nc.sync.dma_start(out=xt_t[:], in_=xt_d[b])
                at_t = stage.tile([128, SPAD], FP32, tag="atload")
                nc.sync.dma_start(out=at_t[:, :S], in_=attn_d[b])
                nc.vector.memset(at_t[:, S:], 0.0)

                # layer norm of x (affine folded into pwt/pb on host)
                st6 = small.tile([128, 6], FP32, tag="st6")
                nc.vector.bn_stats(out=st6[:], in_=x_t[:])
                mv = small.tile([128, 2], FP32, tag="mv")
                nc.vector.bn_aggr(out=mv[:], in_=st6[:])
                sd = small.tile([128, 1], FP32, tag="sd")
                nc.scalar.sqrt(out=sd[:], in_=mv[:, 1:2])
                nc.vector.tensor_scalar_add(out=sd[:], in0=sd[:], scalar1=EPS_LN)
                inv = small.tile([128, 1], FP32, tag="inv")
                nc.vector.reciprocal(out=inv[:], in_=sd[:])
                xn_t = stage.tile([128, D], FP32, tag="xn")
                nc.vector.tensor_scalar(out=xn_t[:], in0=x_t[:],
                                        scalar1=mv[:, 0:1], scalar2=inv[:],
                                        op0=ALU.subtract, op1=ALU.mult)

                # xn^T and attn^T via PE transpose
                for k in range(4):
                    tp = psum_s.tile([128, 128], FP32, space="PSUM", tag="misc")
                    nc.tensor.transpose(out=tp[:], in_=xn_t[:, 128 * k:128 * (k + 1)],
                                        identity=ident[:])
                    nc.vector.tensor_copy(out=xnT[:, k, T * b:T * (b + 1)], in_=tp[:])
                atT_f = stage.tile([128, 4, 128], FP32, tag="atTf")
                for j in range(4):
                    tp = psum_s.tile([128, 128], FP32, space="PSUM", tag="misc")
                    nc.tensor.transpose(out=tp[:], in_=at_t[:, 128 * j:128 * (j + 1)],
                                        identity=ident[:])
                    nc.vector.tensor_copy(out=atT_f[:, j, :], in_=tp[:])
                    nc.vector.tensor_scalar_mul(out=attnT[:, b, j, :], in0=tp[:],
                                                scalar1=4096.0)

                # h_t = attn @ memory, then p_gen gate pieces
                ht = psum.tile([128, D], FP32, space="PSUM", tag="bank")
                for j in range(4):
                    m_t = stage.tile([128, D], FP32, tag="mload")
                    rows = 128 if j < 3 else S - 384
                    nc.sync.dma_start(out=m_t[:rows, :],
                                      in_=mem_d[b, 128 * j:128 * j + rows])
                    nc.tensor.matmul(out=ht[:], lhsT=atT_f[:, j, :], rhs=m_t[:],
                                     start=(j == 0), stop=(j == 3))

                dump = stage.tile([128, D], FP32, tag="xn")
                hw = small.tile([128, 1], FP32, tag="hw")
                nc.vector.tensor_tensor(out=dump[:], in0=ht[:], in1=wh_bc[:], op=ALU.mult)
                nc.vector.tensor_reduce(out=hw[:], in_=dump[:], axis=mybir.AxisListType.X, op=ALU.add)
                xs = small.tile([128, 1], FP32, tag="xs")
                nc.vector.tensor_tensor(out=dump[:], in0=x_t[:], in1=ws_bc[:], op=ALU.mult)
                nc.vector.tensor_reduce(out=xs[:], in_=dump[:], axis=mybir.AxisListType.X, op=ALU.add)
                xts = small.tile([128, 1], FP32, tag="xts")
                nc.vector.tensor_tensor(out=dump[:], in0=xt_t[:], in1=wx_bc[:], op=ALU.mult)
                nc.vector.tensor_reduce(out=xts[:], in_=dump[:], axis=mybir.AxisListType.X, op=ALU.add)

                logit = small.tile([128, 1], FP32, tag="logit")
                nc.vector.tensor_tensor(out=logit[:], in0=hw[:], in1=xs[:], op=ALU.add)
                nc.vector.tensor_tensor(out=logit[:], in0=logit[:], in1=xts[:], op=ALU.add)
                nc.scalar.activation(out=p_all[:, b:b + 1], in_=logit[:],
                                     func=F.Sigmoid, bias=gate_sb[:])
                pc = small.tile([128, 1], FP32, tag="pc")
                nc.vector.tensor_scalar_max(out=pc[:], in0=p_all[:, b:b + 1], scalar1=1e-30)
                nc.scalar.activation(out=lnp[:, b:b + 1], in_=pc[:], func=F.Ln)
                qc = small.tile([128, 1], FP32, tag="qc")
                nc.vector.tensor_scalar(out=qc[:], in0=p_all[:, b:b + 1],
                                        scalar1=-1.0, scalar2=1.0,
                                        op0=ALU.mult, op1=ALU.add)
                nc.vector.tensor_scalar_max(out=qc[:], in0=qc[:], scalar1=1e-30)
                nc.scalar.activation(out=lnq[:, b:b + 1], in_=qc[:], func=F.Ln)
                nc.sync.dma_start(out=pgen_d[b], in_=p_all[:, b:b + 1])

            # ---------- phase 1: dec = xn @ pwt, exp stash + sumexp ----------
            for g2 in range(0, NVT, 2):
                vts = [v for v in (g2, g2 + 1) if v < NVT]
                pw_tiles = {}
                for v in vts:
                    off = VTW * v
                    w = VT_WIDTHS[v]
                    pwt_t = mmp.tile([128, 4, VTW], FP16, tag="pwt")
                    for k in range(4):
                        nc.sync.dma_start(out=pwt_t[:, k, :w],
                                          in_=pwt_d[128 * k:128 * (k + 1), off:off + w])
                    pw_tiles[v] = pwt_t
                for b in range(B):
                    banks = {}
                    for v in vts:
                        banks[v] = psum.tile([128, VTW], FP32, space="PSUM", tag="bank", name=f"bank{v % 2}")
                    for k in range(4):
                        for v in vts:
                            w = VT_WIDTHS[v]
                            nc.tensor.matmul(
                                out=banks[v][:, :w],
                                lhsT=xnT[:, k, T * b:T * (b + 1)],
                                rhs=pw_tiles[v][:, k, :w],
                                start=(k == 0), stop=(k == 3))
                    for v in vts:
                        w = VT_WIDTHS[v]
                        off = VTW * v
                        if projb_nonzero:
                            pbb = stage.tile([128, VTW], FP32, tag="pbb")
                            nc.sync.dma_start(out=pbb[:, :w], in_=bcast(pb_d, w, offset=off))
                            nc.vector.tensor_tensor(out=banks[v][:, :w], in0=banks[v][:, :w],
                                                    in1=pbb[:, :w], op=ALU.add)
                        sacc = small.tile([128, 1], FP32, tag="sacc")
                        nc.scalar.activation(out=a16[:, b, off:off + w], in_=banks[v][:, :w],
                                             func=F.Exp, accum_out=sacc[:])
                        nc.vector.tensor_tensor(out=Z[:, b:b + 1], in0=Z[:, b:b + 1],
                                                in1=sacc[:], op=ALU.add)

            # ---------- AllReduce of sum-exp, then per-token scales ----------
            cc_in = dram.tile([128, B], FP32)
            cc_out = dram.tile([128, B], FP32)
            nc.sync.dma_start(out=cc_in[:], in_=Z[:])
            nc.gpsimd.collective_compute(
                "AllReduce", ALU.add, replica_groups=[list(range(NCORES))],
                ins=[cc_in[:].opt()], outs=[cc_out[:].opt()])
            Zg = singles.tile([128, B], FP32)
            nc.sync.dma_start(out=Zg[:], in_=cc_out[:])

            lnZ = singles.tile([128, B], FP32)
            nc.scalar.activation(out=lnZ[:], in_=Zg[:], func=F.Ln)
            nc.vector.tensor_tensor(out=bias2[:], in0=lnp[:], in1=lnq[:], op=ALU.subtract)
            nc.vector.tensor_tensor(out=bias2[:], in0=bias2[:], in1=lnZ[:], op=ALU.subtract)
            # pz = exp(lnp - lnZ), sinv = exp(-bias2)
            tmp8 = singles.tile([128, B], FP32)
            nc.vector.tensor_tensor(out=tmp8[:], in0=lnp[:], in1=lnZ[:], op=ALU.subtract)
            nc.scalar.activation(out=pz[:], in_=tmp8[:], func=F.Exp)
            # sinv = exp(-bias2)/4096: compensates the 2^12 scaling baked into
            # attnT to keep tiny attention weights in fp16 normal range
            nc.vector.tensor_scalar(out=tmp8[:], in0=bias2[:], scalar1=-1.0,
                                    scalar2=-12.0 * 0.6931471805599453,
                                    op0=ALU.mult, op1=ALU.add)
            nc.scalar.activation(out=sinv[:], in_=tmp8[:], func=F.Exp)

            # ---------- phase 2: mix + log, per (batch, group-of-4-vtiles) ----------
            for b in range(B):
                # sbc[p, t] = sinv[t, b]: transpose the column to a row via PE
                # (out[1,128] = sinv_col.T @ I), then broadcast across partitions
                # with a K=1 ones matmul.
                srow_ps = psum_s.tile([128, 128], FP32, space="PSUM", tag="misc", name="srow_ps")
                nc.tensor.matmul(out=srow_ps[:1, :], lhsT=sinv[:, b:b + 1], rhs=ident[:],
                                 start=True, stop=True)
                srow = small.tile([1, 128], FP32, tag="srow_sb")
                nc.vector.tensor_copy(out=srow[:], in_=srow_ps[:1, :])
                sbc_ps = psum_s.tile([128, 128], FP32, space="PSUM", tag="misc", name="sbc_ps")
                nc.tensor.matmul(out=sbc_ps[:], lhsT=ones_row[:], rhs=srow[:],
                                 start=True, stop=True)
                sbc = small.tile([128, 128], FP32, tag="sbc_sb")
                nc.vector.tensor_copy(out=sbc[:], in_=sbc_ps[:])

                for g in range(NGRP):
                    goff = 2048 * g
                    vts = [v for v in range(GRP * g, min(GRP * (g + 1), NVT))]
                    cw = sum(VT_WIDTHS[v] for v in vts)

                    g16_t = p2s.tile([128, 4, GRP * RSLOT], FP16, tag="g16")
                    nc.sync.dma_start(out=g16_t[:], in_=g16_d[b, g])
                    sel_t = p2p.tile([RSLOT, GRP, VTW], BF16, tag="sel")
                    nc.sync.dma_start(out=sel_t[:], in_=sel_d[b, g])

                    # M1: compact^T [slots, t] = G16^T @ attn^T
                    cT_ps = psum_s.tile([GRP * RSLOT, 128], FP32, space="PSUM", tag="misc", name="cT_ps")
                    for k in range(4):
                        nc.tensor.matmul(out=cT_ps[:], lhsT=g16_t[:, k, :],
                                         rhs=attnT[:, b, k, :],
                                         start=(k == 0), stop=(k == 3))
                    # scale by sinv[t] during PSUM->SBUF copy (t is the free axis);
                    # split to 4 x [32,128] so M2 operands start at partition 0
                    cT_sb = p2s.tile([RSLOT, GRP, 128], BF16, tag="cTsb")
                    for vi in range(GRP):
                        nc.vector.tensor_tensor(
                            out=cT_sb[:, vi, :],
                            in0=cT_ps[RSLOT * vi:RSLOT * (vi + 1), :],
                            in1=sbc[:RSLOT, :], op=ALU.mult)

                    # m = exp(dec) + enc', per vocab tile (enc' carries q/pz);
                    # then out = ln(pz*m + eps) in place.
                    A_t = p2p.tile([128, 2048], FP32, tag="A")
                    for vi, v in enumerate(vts):
                        w = VT_WIDTHS[v]
                        enc_ps = psum_e.tile([128, VTW], FP32, space="PSUM", tag="enc")
                        nc.tensor.matmul(out=enc_ps[:, :w],
                                         lhsT=cT_sb[:, vi, :],
                                         rhs=sel_t[:, vi, :w],
                                         start=True, stop=True)
                        sl = slice(VTW * vi, VTW * vi + w)
                        nc.vector.tensor_tensor(out=A_t[:, sl],
                                                in0=a16[:, b, goff + VTW * vi:goff + VTW * vi + w],
                                                in1=enc_ps[:, :w], op=ALU.add)
                    nc.scalar.activation(out=A_t[:, :cw], in_=A_t[:, :cw], func=F.Ln,
                                         scale=pz[:, b:b + 1], bias=eps_sb[:])
                    nc.sync.dma_start(out=out_d[b][:, goff:goff + cw], in_=A_t[:, :cw])

    _split_big_waits(nc)
    return nc


def _host_prep(inputs):
    x = np.ascontiguousarray(inputs["x"], np.float32)
    x_t = np.ascontiguousarray(inputs["x_t"], np.float32)
    memory = np.ascontiguousarray(inputs["memory"], np.float32)
    attn = np.ascontiguousarray(inputs["attn_weights"], np.float32)
    src = np.asarray(inputs["src"])
    norm_a = np.asarray(inputs["norm_a"], np.float32)
    norm_b = np.asarray(inputs["norm_b"], np.float32)
    pw = np.asarray(inputs["proj_w"], np.float32)
    pb = np.asarray(inputs["proj_b"], np.float32)

    # fold LN affine into the projection:  dec = LNraw(x) @ (a*pw)^T + (pb + pw@b)
    pw_eff_T = (pw * norm_a[None, :]).T          # [D, V]
    pb_eff = pb + pw @ norm_b                    # [V]
    projb_nonzero = bool(np.any(pb_eff != 0.0))

    gatec = np.array(
        [float(inputs["wh_b"][0]) + float(inputs["ws_b"][0])
         + float(inputs["wx_b"][0]) + float(inputs["bptr"][0, 0])], np.float32)
    identf = np.eye(128, dtype=np.float32)

    # per-core selection matrices for the copy distribution
    g16_all, sel_all, pwt_all, pb_all = [], [], [], []
    for c in range(NCORES):
        lo = c * VS
        g16 = np.zeros((B, NGRP, 128, 4, GRP * RSLOT), np.float32)
        sel = np.zeros((B, NGRP, RSLOT, GRP, VTW), np.float32)
        for b in range(B):
            ids = src[b]
            msk = (ids >= lo) & (ids < lo + VS)
            s_idx = np.nonzero(msk)[0]
            loc = ids[s_idx] - lo
            for g in range(NGRP):
                vts = range(GRP * g, min(GRP * (g + 1), NVT))
                for vi, v in enumerate(vts):
                    vlo, vhi = VTW * v, VTW * v + VT_WIDTHS[v]
                    sub = (loc >= vlo) & (loc < vhi)
                    if not np.any(sub):
                        continue
                    ss = s_idx[sub]
                    ll = loc[sub] - vlo
                    uniq, inv = np.unique(ll, return_inverse=True)
                    if len(uniq) > RSLOT:
                        raise RuntimeError(
                            f"slot overflow: {len(uniq)} > {RSLOT}")
                    col = RSLOT * vi + inv
                    g16[b, g, ss % 128, ss // 128, col] = 1.0
                    sel[b, g, np.arange(len(uniq)), vi, uniq] = 1.0
        g16_all.append(g16.astype(np.float32))
        sel_all.append(sel.astype(np.float32))
        pwt_all.append(np.ascontiguousarray(pw_eff_T[:, lo:lo + VS]))
        pb_all.append(np.ascontiguousarray(pb_eff[lo:lo + VS]))

    from ml_dtypes import bfloat16 as _bf16
    in_maps = []
    for c in range(NCORES):
        in_maps.append({
            "x": x, "x_t": x_t, "memory": memory, "attn": attn,
            "pwt": pwt_all[c].astype(np.float16), "pb": pb_all[c],
            "whv": np.ascontiguousarray(inputs["wh_w"][0], np.float32),
            "wsv": np.ascontiguousarray(inputs["ws_w"][0], np.float32),
            "wxv": np.ascontiguousarray(inputs["wx_w"][0], np.float32),
            "gatec": gatec, "identf": identf,
            "g16": g16_all[c].astype(np.float16),
            "sel16": sel_all[c].astype(_bf16),
        })
    return in_maps, projb_nonzero


def kernel(**inputs):
    global last_results
    in_maps, projb_nonzero = _host_prep(inputs)

    key = ("prog", projb_nonzero)
    if key not in _CACHE:
        _CACHE[key] = _build_program(projb_nonzero)
    nc = _CACHE[key]

    trace = bool(os.environ.get("TRN_KERNEL_TRACE"))
    res = run_bass_kernel_spmd(nc, in_maps, list(range(NCORES)), trace=trace)
    last_results = res

    out = np.empty((B, T, V), np.float32)
    for c in range(NCORES):
        out[:, :, c * VS:(c + 1) * VS] = res.results[c]["out"]
    p_gen = np.ascontiguousarray(res.results[0]["pgen"], np.float32)
    return out, p_gen
